# revision 6
# baseline (speedup 1.0000x reference)
"""Tensor-parallel causal multi-head attention for 8 TRN2 NeuronCores — v2.

Problem: B=2, T=2048, HIDDEN=2048, 16 heads x 128 head_dim, causal, RoPE.
Sharding: 2 heads per core (tensor parallel). Each core computes its QKV
projections, RoPE, causal attention, and a partial output projection over
its 256 hidden features; the host sums the 8 partial outputs.

v2 vs v1: zero PE transposes.
 - Q^T/K^T are produced directly in [d, t] layout by making the weight
   c-chunk the matmul stationary operand and streaming tokens.
 - A per-head d-permutation (quadrant-local [16 even | 16 odd] pairs)
   makes RoPE applicable in [d, t] layout: the even/odd partner lives 16
   partitions away within the same 32-partition quadrant, reachable by
   DVE stream_shuffle. Scores are permutation-invariant since Q and K
   share the permutation.
 - V is produced directly in [t, d] layout (x tt-chunk stationary,
   wv columns streaming), which is what the PV matmul needs.
 - The softmax-denominator reciprocal broadcast runs on the idle Pool
   engine (partition_broadcast) instead of a PE matmul + DVE copy.

Device compute dtype: bf16 matmuls with f32 PSUM accumulation; softmax in
f32 (no max-subtraction needed: |scores/sqrt(d)| < ~8 for this data scale).

Layouts (per core):
  xt    [2048 c, 4096 t]  bf16   (x transposed; contraction dim on partitions)
  wqt/wkt [2048 c, 256 d] bf16   (head-slice of wq/wk, rows d-permuted per
                                  head by RPERM, then transposed)
  wvt   [2048 c, 256 d]  bf16   (natural d order)
  wot   [256 c, 2048 d]  bf16   (per-core row-slice of wo.T, natural order)
  ct/st [128 p, 2048 t]  f32    (cos/sin rows matched to the d-permutation;
                                 st's top 16 rows of each quadrant negated)
  out   [4096 t, 2048 d]  bf16   partial output (host sums over cores)
"""

import numpy as np
import ml_dtypes
from contextlib import ExitStack

import concourse.bass as bass
import concourse.mybir as mybir
import concourse.tile as tile
from concourse import bacc
from concourse.bass_utils import run_bass_kernel_spmd

F32 = mybir.dt.float32
BF16 = mybir.dt.bfloat16

NCORES = 8
B, T, C = 2, 2048, 2048
TT = B * T              # 4096 flattened rows
NH, D = 16, 128         # global heads, head dim
HL = NH // NCORES       # 2 local heads
DH = HL * D             # 256 local head features
NE = 8                  # t-eighths of 512 rows
ET = TT // NE           # 512 rows per eighth
CT = C // 128           # 16 contraction tiles
SCALE = 1.0 / float(np.sqrt(D))

# quadrant-local even/odd permutation: partition p (within a head's 128)
# holds original d = RPERM[p]; pairs (2j, 2j+1) sit 16 partitions apart
# within one 32-partition quadrant so stream_shuffle can swap them.
RPERM = np.array(
    [2 * (16 * (p // 32) + p % 16) + (0 if p % 32 < 16 else 1) for p in range(128)]
)
SHUF = [(i + 16) % 32 for i in range(32)]

_CACHE: dict = {}


def _build(T=T, B=B, num_devices=NCORES, repeat=1, small_out=False,
           stop_after=None, ablate=(), bcast="pool"):
    TT = B * T
    NE = TT // 512
    ET = 512
    nc = bacc.Bacc("TRN2", target_bir_lowering=False, debug=False,
                   num_devices=num_devices)
    xt = nc.dram_tensor("xt", [C, TT], BF16, kind="ExternalInput").ap()
    wqt = nc.dram_tensor("wqt", [C, DH], BF16, kind="ExternalInput").ap()
    wkt = nc.dram_tensor("wkt", [C, DH], BF16, kind="ExternalInput").ap()
    wvt = nc.dram_tensor("wvt", [C, DH], BF16, kind="ExternalInput").ap()
    wot = nc.dram_tensor("wot", [DH, C], BF16, kind="ExternalInput").ap()
    ct = nc.dram_tensor("ct", [128, T], F32, kind="ExternalInput").ap()
    st = nc.dram_tensor("st", [128, T], F32, kind="ExternalInput").ap()
    out = nc.dram_tensor("out", [128 if small_out else TT, C], BF16,
                         kind="ExternalOutput").ap()

    with ExitStack() as ctx:
        tc = ctx.enter_context(tile.TileContext(nc))
        # ---- persistent tiles -------------------------------------------
        gp = ctx.enter_context(tc.tile_pool(name="glob", bufs=1))
        wq_sb = gp.tile([128, CT * DH], BF16)
        wk_sb = gp.tile([128, CT * DH], BF16)
        wv_sb = gp.tile([128, CT * DH], BF16)
        wo_sb = gp.tile([128, HL * C], BF16)
        # load order matters at startup: wq is needed first (Q block of
        # eighth 0), wo not until phase 2. Alternate queues for overlap.
        for i, (dst, src_ap, nd) in enumerate(
                ((wq_sb, wqt, DH), (wk_sb, wkt, DH), (wv_sb, wvt, DH),
                 (wo_sb, wot, C))):
            eng = nc.sync if i % 2 == 0 else nc.scalar
            eng.dma_start(
                dst[:].rearrange("p (k d) -> p k d", d=nd),
                src_ap.rearrange("(k p) d -> p k d", p=128))
        ct_sb = gp.tile([128, T], F32)
        st_sb = gp.tile([128, T], F32)
        nc.scalar.dma_start(ct_sb[:], ct)
        nc.sync.dma_start(st_sb[:], st)

        v_all = gp.tile([128, (TT // 128) * DH], BF16)   # [t-in-tile, g*DH+d]
        qT = [gp.tile([128, TT], BF16, name=f"qT{h}") for h in range(HL)]
        kT = [gp.tile([128, TT], BF16, name=f"kT{h}") for h in range(HL)]

        ones_col = gp.tile([128, 1], BF16)
        nc.vector.memset(ones_col[:], 1.0)
        ones_row = gp.tile([1, 128], F32)
        nc.vector.memset(ones_row[:], 1.0)

        # static causal masks for the 4 diagonal block offsets:
        # mask_k keeps [p, x] iff x >= 128k + p
        pairmasks = []
        mtmp = gp.tile([128, 512], F32)
        for m in range(2):
            pm = gp.tile([128, 1024], BF16, name=f"pmask{m}")
            for half in range(2):
                k = 2 * m + half
                nc.vector.memset(mtmp[:], 1.0)
                nc.gpsimd.affine_select(
                    out=mtmp[:], in_=mtmp[:],
                    compare_op=mybir.AluOpType.is_ge, fill=0.0,
                    base=-128 * k, pattern=[[1, 512]], channel_multiplier=-1,
                )
                nc.vector.tensor_copy(pm[:, half * 512:(half + 1) * 512], mtmp[:])
            pairmasks.append(pm)

        def rope(rp, src, dstl, t0, trow):
            # src: PSUM [128, HL*512] f32 (per-head banks); dst: qT/kT slices
            for h in range(HL):
                ph = src[:, h * 512:(h + 1) * 512]
                rsh = rp.tile([128, 512], F32, tag="rsh")
                nc.vector.stream_shuffle(rsh[:], ph, SHUF)
                t2 = rp.tile([128, 512], F32, tag="t2")
                nc.vector.tensor_mul(t2[:], ph, ct_sb[:, trow:trow + ET])
                t1 = rp.tile([128, 512], F32, tag="t1")
                nc.vector.tensor_mul(t1[:], rsh[:], st_sb[:, trow:trow + ET])
                nc.vector.tensor_add(dstl[h][:, t0:t0 + ET], t2[:], t1[:])

        for _rep in range(repeat):
         # ---- phase 1: QKV projections + RoPE (no transposes) ------------
         with ExitStack() as p1:
            xp = p1.enter_context(tc.tile_pool(name="xin", bufs=32))
            rp = p1.enter_context(tc.tile_pool(name="rtmp", bufs=2))
            pq = p1.enter_context(tc.tile_pool(name="pq", bufs=1, space="PSUM"))
            pk = p1.enter_context(tc.tile_pool(name="pk", bufs=1, space="PSUM"))
            pv = p1.enter_context(tc.tile_pool(name="pv", bufs=1, space="PSUM"))

            for e in range(NE):
                t0 = e * ET
                trow = t0 % T
                xcs = []
                for c in range(CT):
                    xc = xp.tile([128, ET], BF16, tag="xc")
                    dma_eng = nc.sync if c % 2 == 0 else nc.scalar
                    dma_eng.dma_start(xc[:], xt[c * 128:(c + 1) * 128, t0:t0 + ET])
                    xcs.append(xc)

                pQ = pq.tile([128, HL * 512], F32, tag="pQ")
                pK = pk.tile([128, HL * 512], F32, tag="pK")
                pV = [pv.tile([128, 512], F32, tag=f"pV{i}", name=f"pV{i}")
                      for i in range(2)]

                # Q block: weight chunk stationary, tokens streaming
                for c in range(CT):
                    for h in range(HL):
                        nc.tensor.matmul(
                            pQ[:, h * 512:(h + 1) * 512],
                            wq_sb[:, c * DH + h * 128: c * DH + (h + 1) * 128],
                            xcs[c][:], start=(c == 0), stop=(c == CT - 1))
                rope(rp, pQ[:], qT, t0, trow)

                # K block
                for c in range(CT):
                    for h in range(HL):
                        nc.tensor.matmul(
                            pK[:, h * 512:(h + 1) * 512],
                            wk_sb[:, c * DH + h * 128: c * DH + (h + 1) * 128],
                            xcs[c][:], start=(c == 0), stop=(c == CT - 1))
                rope(rp, pK[:], kT, t0, trow)

                # V block: x tt-chunk stationary, wv columns streaming.
                # Two tt-chunks share one PSUM bank; start=True clears the
                # WHOLE bank, so only the bank's first matmul may set it —
                # the odd tt's first write lands via per-element has_written
                # overwrite semantics.
                for c in range(CT):
                    for tt in range(4):
                        nc.tensor.matmul(
                            pV[tt // 2][:, (tt % 2) * 256:(tt % 2) * 256 + 256],
                            xcs[c][:, tt * 128:(tt + 1) * 128],
                            wv_sb[:, c * DH:(c + 1) * DH],
                            start=(c == 0 and tt % 2 == 0),
                            stop=(c == CT - 1 and tt % 2 == 1))
                for tt in range(4):
                    g = t0 // 128 + tt
                    nc.scalar.copy(
                        v_all[:, g * DH:(g + 1) * DH],
                        pV[tt // 2][:, (tt % 2) * 256:(tt % 2) * 256 + 256])

         if stop_after == "qkv":
             with tc.tile_pool(name="dump", bufs=1) as dump:
                 dt_ = dump.tile([128, TT], F32, name="dt_")
                 nc.vector.tensor_copy(dt_[:], qT[0][:])
                 nc.vector.tensor_add(dt_[:], dt_[:], kT[1][:])
                 nc.vector.tensor_add(dt_[:], dt_[:], v_all[:, 0:TT])
                 nc.sync.dma_start(out[0:128, 0:C], dt_[:, 0:C])
                 nc.vector.tensor_copy(dt_[:], qT[1][:])
                 nc.vector.tensor_add(dt_[:], dt_[:], kT[0][:])
                 nc.sync.dma_start(out[0:128, 0:C], dt_[:, 0:C])
             continue

         # ---- phase 2: attention + output projection ---------------------
         with ExitStack() as p2:
             ptp = p2.enter_context(tc.tile_pool(name="ptile", bufs=16))
             atp = p2.enter_context(tc.tile_pool(name="attnT", bufs=6))
             rdp = p2.enter_context(tc.tile_pool(name="rden", bufs=2))
             osp = p2.enter_context(tc.tile_pool(name="ost", bufs=4))
             psw = p2.enter_context(tc.tile_pool(name="psw", bufs=2, space="PSUM"))
             pso = p2.enter_context(tc.tile_pool(name="pso", bufs=2, space="PSUM"))
             psa = p2.enter_context(tc.tile_pool(name="psa", bufs=1, space="PSUM"))
             psd = p2.enter_context(tc.tile_pool(name="psd", bufs=1, space="PSUM"))

             def oproj(attnT, q0):
                 # output projection for a finished group — issued one group
                 # late so its (ready) matmuls fill PE gaps while the next
                 # group's softmax chain is still in flight.
                 for tt in range(4):
                     r0 = q0 + tt * 128
                     ost = osp.tile([128, C], BF16, tag="ost")
                     for oc in range(4):
                         pO = pso.tile([128, 512], F32, tag="pso")
                         for h in range(HL):
                             nc.tensor.matmul(
                                 pO[:], attnT[h][:, tt * 128:(tt + 1) * 128],
                                 wo_sb[:, h * C + oc * 512: h * C + oc * 512 + 512],
                                 start=(h == 0), stop=(h == HL - 1))
                         nc.vector.tensor_copy(ost[:, oc * 512:(oc + 1) * 512],
                                               pO[:])
                     if small_out:
                         nc.scalar.dma_start(out[0:128, :], ost[:])
                     else:
                         nc.scalar.dma_start(out[r0:r0 + 128, :], ost[:])

             pending = None
             for b in range(B):
                 for j in range(T // 512):   # q-chunks of 512 within the batch
                     q0 = b * T + j * 512
                     nkt = 4 * j + 4
                     npair = nkt // 2
                     # diagonal (masked) pairs FIRST: their exp->mask->PV
                     # chain is longest; later unmasked pairs keep the PE fed
                     # while it completes.
                     porder = [2 * j, 2 * j + 1] + list(range(2 * j))
                     attnT = []
                     for h in range(HL):
                         pA = psa.tile([128, 512], F32, tag="psa")
                         pDen = psd.tile([1, 512], F32, tag="psd")
                         for pi, p_ in enumerate(porder):
                             pS = psw.tile([128, 1024], F32, tag="psw")
                             ptile = ptp.tile([128, 1024], BF16, tag="ptile")
                             for half in range(2):
                                 i = 2 * p_ + half
                                 g = b * (T // 128) + i
                                 nc.tensor.matmul(
                                     pS[:, half * 512:(half + 1) * 512],
                                     kT[h][:, g * 128:(g + 1) * 128],
                                     qT[h][:, q0:q0 + 512], start=True, stop=True)
                             nc.scalar.activation(
                                 ptile[:], pS[:],
                                 mybir.ActivationFunctionType.Exp, scale=SCALE)
                             if 2 * p_ >= 4 * j and "nomask" not in ablate:
                                 nc.vector.tensor_mul(
                                     ptile[:], ptile[:], pairmasks[p_ - 2 * j][:])
                             for half in range(2):
                                 g = b * (T // 128) + 2 * p_ + half
                                 pt_h = ptile[:, half * 512:(half + 1) * 512]
                                 if "nopv" not in ablate:
                                     nc.tensor.matmul(
                                         pA[:],
                                         v_all[:, g * DH + h * 128: g * DH + (h + 1) * 128],
                                         pt_h, start=(pi == 0 and half == 0),
                                         stop=(pi == npair - 1 and half == 1))
                             # pre-add the two halves -> ONE den matmul per pair
                             pds = rdp.tile([128, 512], BF16, tag="pds")
                             nc.vector.tensor_add(
                                 pds[:], ptile[:, 0:512], ptile[:, 512:1024])
                             nc.tensor.matmul(
                                 pDen[:], ones_col[:], pds[:],
                                 start=(pi == 0), stop=(pi == npair - 1))
                         rden = rdp.tile([1, 512], F32, tag="rden")
                         nc.vector.reciprocal(rden[:], pDen[:])
                         bc = rdp.tile([128, 512], F32, tag="bc")
                         if bcast == "pool":
                             nc.gpsimd.partition_broadcast(bc[:], rden[:])
                         else:
                             pB = pso.tile([128, 512], F32, tag="pso")
                             nc.tensor.matmul(pB[:], ones_row[:], rden[:],
                                              start=True, stop=True)
                             nc.vector.tensor_copy(bc[:], pB[:])
                         aT = atp.tile([128, 512], BF16, tag=f"aT{h}")
                         nc.vector.tensor_mul(aT[:], pA[:], bc[:])
                         attnT.append(aT)

                     if stop_after == "attn":
                         ost = osp.tile([128, C], F32, tag="ost", name="osta")
                         nc.vector.tensor_copy(ost[:, 0:512], attnT[0][:])
                         nc.vector.tensor_copy(ost[:, 512:1024], attnT[1][:])
                         nc.sync.dma_start(out[0:128, 0:1024], ost[:, 0:1024])
                         continue
                     if pending is not None:
                         oproj(*pending)
                     pending = (attnT, q0)
             if pending is not None and stop_after != "attn":
                 oproj(*pending)

    nc.compile()
    return nc


def _get_nc():
    if "nc" not in _CACHE:
        _CACHE["nc"] = _build()
    return _CACHE["nc"]


def _trig_tiles(freqs_cos, freqs_sin):
    cos = np.asarray(freqs_cos, np.float32)   # [T, 64]
    sin = np.asarray(freqs_sin, np.float32)
    jrow = np.array([16 * (p // 32) + p % 16 for p in range(128)])
    sign = np.array([-1.0 if p % 32 < 16 else 1.0 for p in range(128)],
                    np.float32)
    ct = np.ascontiguousarray(cos.T[jrow])
    st = np.ascontiguousarray(sin.T[jrow] * sign[:, None])
    return ct, st


def kernel(x, wq, wk, wv, wo, freqs_cos, freqs_sin, mask=None, **_unused):
    bf = ml_dtypes.bfloat16
    nc = _get_nc()

    x = np.asarray(x, dtype=np.float32)
    xt = np.ascontiguousarray(x.reshape(TT, C).T).astype(bf)
    ct, st = _trig_tiles(freqs_cos, freqs_sin)
    perm_local = np.concatenate([h * 128 + RPERM for h in range(HL)])

    in_maps = []
    for i in range(NCORES):
        sl = slice(DH * i, DH * (i + 1))
        wq_l = np.asarray(wq, np.float32)[sl, :][perm_local, :]
        wk_l = np.asarray(wk, np.float32)[sl, :][perm_local, :]
        in_maps.append({
            "xt": xt,
            "wqt": np.ascontiguousarray(wq_l.T).astype(bf),
            "wkt": np.ascontiguousarray(wk_l.T).astype(bf),
            "wvt": np.ascontiguousarray(np.asarray(wv, np.float32)[sl, :].T).astype(bf),
            "wot": np.ascontiguousarray(np.asarray(wo, np.float32)[:, sl].T).astype(bf),
            "ct": ct,
            "st": st,
        })

    res = run_bass_kernel_spmd(nc, in_maps, core_ids=list(range(NCORES)))
    acc = np.zeros((TT, C), dtype=np.float32)
    for r in res.results:
        acc += np.asarray(r["out"], dtype=np.float32)
    return acc.reshape(B, T, C)


# revision 12
# speedup vs baseline: 1.3099x; 1.3099x over previous
"""Tensor-parallel causal multi-head attention for 8 TRN2 NeuronCores — v2.

Problem: B=2, T=2048, HIDDEN=2048, 16 heads x 128 head_dim, causal, RoPE.
Sharding: 2 heads per core (tensor parallel). Each core computes its QKV
projections, RoPE, causal attention, and a partial output projection over
its 256 hidden features; the host sums the 8 partial outputs.

v2 vs v1: zero PE transposes.
 - Q^T/K^T are produced directly in [d, t] layout by making the weight
   c-chunk the matmul stationary operand and streaming tokens.
 - A per-head d-permutation (quadrant-local [16 even | 16 odd] pairs)
   makes RoPE applicable in [d, t] layout: the even/odd partner lives 16
   partitions away within the same 32-partition quadrant, reachable by
   DVE stream_shuffle. Scores are permutation-invariant since Q and K
   share the permutation.
 - V is produced directly in [t, d] layout (x tt-chunk stationary,
   wv columns streaming), which is what the PV matmul needs.
 - The softmax-denominator reciprocal broadcast runs on the idle Pool
   engine (partition_broadcast) instead of a PE matmul + DVE copy.

Device compute dtype: bf16 matmuls with f32 PSUM accumulation; softmax in
f32 (no max-subtraction needed: |scores/sqrt(d)| < ~8 for this data scale).

Layouts (per core):
  xt    [2048 c, 4096 t]  bf16   (x transposed; contraction dim on partitions)
  wqt/wkt [2048 c, 256 d] bf16   (head-slice of wq/wk, rows d-permuted per
                                  head by RPERM, then transposed)
  wvt   [2048 c, 256 d]  bf16   (natural d order)
  wot   [256 c, 2048 d]  bf16   (per-core row-slice of wo.T, natural order)
  ct/st [128 p, 2048 t]  f32    (cos/sin rows matched to the d-permutation;
                                 st's top 16 rows of each quadrant negated)
  out   [4096 t, 2048 d]  bf16   partial output (host sums over cores)
"""

import numpy as np
import ml_dtypes
from contextlib import ExitStack

import concourse.bass as bass
import concourse.mybir as mybir
import concourse.tile as tile
from concourse import bacc
from concourse.bass_utils import run_bass_kernel_spmd

F32 = mybir.dt.float32
BF16 = mybir.dt.bfloat16

NCORES = 8
B, T, C = 2, 2048, 2048
TT = B * T              # 4096 flattened rows
NH, D = 16, 128         # global heads, head dim
HL = NH // NCORES       # 2 local heads
DH = HL * D             # 256 local head features
NE = 8                  # t-eighths of 512 rows
ET = TT // NE           # 512 rows per eighth
CT = C // 128           # 16 contraction tiles
SCALE = 1.0 / float(np.sqrt(D))

# quadrant-local even/odd permutation: partition p (within a head's 128)
# holds original d = RPERM[p]; pairs (2j, 2j+1) sit 16 partitions apart
# within one 32-partition quadrant so stream_shuffle can swap them.
RPERM = np.array(
    [2 * (16 * (p // 32) + p % 16) + (0 if p % 32 < 16 else 1) for p in range(128)]
)
SHUF = [(i + 16) % 32 for i in range(32)]

_CACHE: dict = {}


def _build(T=T, B=B, num_devices=NCORES, repeat=1, small_out=False,
           stop_after=None, ablate=(), bcast="pool"):
    TT = B * T
    NE = TT // 512
    ET = 512
    nc = bacc.Bacc("TRN2", target_bir_lowering=False, debug=False,
                   num_devices=num_devices)
    xt = nc.dram_tensor("xt", [C, TT], BF16, kind="ExternalInput").ap()
    wqt = nc.dram_tensor("wqt", [C, DH], BF16, kind="ExternalInput").ap()
    wkt = nc.dram_tensor("wkt", [C, DH], BF16, kind="ExternalInput").ap()
    wvt = nc.dram_tensor("wvt", [C, DH], BF16, kind="ExternalInput").ap()
    wot = nc.dram_tensor("wot", [DH, C], BF16, kind="ExternalInput").ap()
    ct = nc.dram_tensor("ct", [128, T], F32, kind="ExternalInput").ap()
    st = nc.dram_tensor("st", [128, T], F32, kind="ExternalInput").ap()
    out = nc.dram_tensor("out", [128 if small_out else TT, C], BF16,
                         kind="ExternalOutput").ap()

    with ExitStack() as ctx:
        tc = ctx.enter_context(tile.TileContext(nc))
        # ---- persistent tiles -------------------------------------------
        gp = ctx.enter_context(tc.tile_pool(name="glob", bufs=1))
        wq_sb = gp.tile([128, CT * DH], BF16)
        wk_sb = gp.tile([128, CT * DH], BF16)
        wv_sb = gp.tile([128, CT * DH], BF16)
        wo_sb = gp.tile([128, HL * C], BF16)
        # load order matters at startup: wq is needed first (Q block of
        # eighth 0), wo not until phase 2. Alternate queues for overlap.
        for i, (dst, src_ap, nd) in enumerate(
                ((wq_sb, wqt, DH), (wk_sb, wkt, DH), (wv_sb, wvt, DH),
                 (wo_sb, wot, C))):
            eng = nc.sync if i % 2 == 0 else nc.scalar
            eng.dma_start(
                dst[:].rearrange("p (k d) -> p k d", d=nd),
                src_ap.rearrange("(k p) d -> p k d", p=128))
        ct_sb = gp.tile([128, T], F32)
        st_sb = gp.tile([128, T], F32)
        nc.scalar.dma_start(ct_sb[:], ct)
        nc.sync.dma_start(st_sb[:], st)

        v_all = gp.tile([128, (TT // 128) * DH], BF16)   # [t-in-tile, g*DH+d]
        qT = [gp.tile([128, TT], BF16, name=f"qT{h}") for h in range(HL)]
        kT = [gp.tile([128, TT], BF16, name=f"kT{h}") for h in range(HL)]

        # all-ones stationary: den-matmul output rows all equal the column
        # sums, i.e. the softmax denominator arrives pre-broadcast across
        # partitions -- no separate broadcast step needed.
        ones_mat = gp.tile([128, 128], BF16)
        nc.vector.memset(ones_mat[:], 1.0)

        # static causal masks for the 4 diagonal block offsets:
        # mask_k keeps [p, x] iff x >= 128k + p
        pairmasks = []
        mtmp = gp.tile([128, 512], F32)
        for m in range(2):
            pm = gp.tile([128, 1024], BF16, name=f"pmask{m}")
            for half in range(2):
                k = 2 * m + half
                nc.vector.memset(mtmp[:], 1.0)
                nc.gpsimd.affine_select(
                    out=mtmp[:], in_=mtmp[:],
                    compare_op=mybir.AluOpType.is_ge, fill=0.0,
                    base=-128 * k, pattern=[[1, 512]], channel_multiplier=-1,
                )
                nc.vector.tensor_copy(pm[:, half * 512:(half + 1) * 512], mtmp[:])
            pairmasks.append(pm)

        def rope(rp, src, dstl, t0, trow):
            # src: PSUM [128, HL*512] f32 (per-head banks); dst: qT/kT slices
            for h in range(HL):
                ph = src[:, h * 512:(h + 1) * 512]
                rsh = rp.tile([128, 512], F32, tag="rsh")
                nc.vector.stream_shuffle(rsh[:], ph, SHUF)
                t2 = rp.tile([128, 512], F32, tag="t2")
                nc.vector.tensor_mul(t2[:], ph, ct_sb[:, trow:trow + ET])
                t1 = rp.tile([128, 512], F32, tag="t1")
                nc.vector.tensor_mul(t1[:], rsh[:], st_sb[:, trow:trow + ET])
                nc.vector.tensor_add(dstl[h][:, t0:t0 + ET], t2[:], t1[:])

        for _rep in range(repeat):
         # ---- phase 1: QKV projections + RoPE (no transposes) ------------
         with ExitStack() as p1:
            xp = p1.enter_context(tc.tile_pool(name="xin", bufs=32))
            rp = p1.enter_context(tc.tile_pool(name="rtmp", bufs=2))
            pq = p1.enter_context(tc.tile_pool(name="pq", bufs=1, space="PSUM"))
            pk = p1.enter_context(tc.tile_pool(name="pk", bufs=1, space="PSUM"))
            pv = p1.enter_context(tc.tile_pool(name="pv", bufs=1, space="PSUM"))

            for e in range(NE):
                t0 = e * ET
                trow = t0 % T
                xcs = []
                for c in range(CT):
                    xc = xp.tile([128, ET], BF16, tag="xc")
                    dma_eng = nc.sync if c % 2 == 0 else nc.scalar
                    dma_eng.dma_start(xc[:], xt[c * 128:(c + 1) * 128, t0:t0 + ET])
                    xcs.append(xc)

                pQ = pq.tile([128, HL * 512], F32, tag="pQ")
                pK = pk.tile([128, HL * 512], F32, tag="pK")
                pV = [pv.tile([128, 512], F32, tag=f"pV{i}", name=f"pV{i}")
                      for i in range(2)]

                # Q block: weight chunk stationary, tokens streaming
                for c in range(CT):
                    for h in range(HL):
                        nc.tensor.matmul(
                            pQ[:, h * 512:(h + 1) * 512],
                            wq_sb[:, c * DH + h * 128: c * DH + (h + 1) * 128],
                            xcs[c][:], start=(c == 0), stop=(c == CT - 1))
                rope(rp, pQ[:], qT, t0, trow)

                # K block
                for c in range(CT):
                    for h in range(HL):
                        nc.tensor.matmul(
                            pK[:, h * 512:(h + 1) * 512],
                            wk_sb[:, c * DH + h * 128: c * DH + (h + 1) * 128],
                            xcs[c][:], start=(c == 0), stop=(c == CT - 1))
                rope(rp, pK[:], kT, t0, trow)

                # V block: x tt-chunk stationary, wv columns streaming.
                # Two tt-chunks share one PSUM bank; start=True clears the
                # WHOLE bank, so only the bank's first matmul may set it —
                # the odd tt's first write lands via per-element has_written
                # overwrite semantics.
                for c in range(CT):
                    for tt in range(4):
                        nc.tensor.matmul(
                            pV[tt // 2][:, (tt % 2) * 256:(tt % 2) * 256 + 256],
                            xcs[c][:, tt * 128:(tt + 1) * 128],
                            wv_sb[:, c * DH:(c + 1) * DH],
                            start=(c == 0 and tt % 2 == 0),
                            stop=(c == CT - 1 and tt % 2 == 1))
                for tt in range(4):
                    g = t0 // 128 + tt
                    nc.scalar.copy(
                        v_all[:, g * DH:(g + 1) * DH],
                        pV[tt // 2][:, (tt % 2) * 256:(tt % 2) * 256 + 256])

         if stop_after == "qkv":
             with tc.tile_pool(name="dump", bufs=1) as dump:
                 dt_ = dump.tile([128, TT], F32, name="dt_")
                 nc.vector.tensor_copy(dt_[:], qT[0][:])
                 nc.vector.tensor_add(dt_[:], dt_[:], kT[1][:])
                 nc.vector.tensor_add(dt_[:], dt_[:], v_all[:, 0:TT])
                 nc.sync.dma_start(out[0:128, 0:C], dt_[:, 0:C])
                 nc.vector.tensor_copy(dt_[:], qT[1][:])
                 nc.vector.tensor_add(dt_[:], dt_[:], kT[0][:])
                 nc.sync.dma_start(out[0:128, 0:C], dt_[:, 0:C])
             continue

         # ---- phase 2: attention + output projection ---------------------
         with ExitStack() as p2:
             ptp = p2.enter_context(tc.tile_pool(name="ptile", bufs=16))
             atp = p2.enter_context(tc.tile_pool(name="attnT", bufs=6))
             rdp = p2.enter_context(tc.tile_pool(name="rden", bufs=2))
             osp = p2.enter_context(tc.tile_pool(name="ost", bufs=4))
             psw = p2.enter_context(tc.tile_pool(name="psw", bufs=2, space="PSUM"))
             pso = p2.enter_context(tc.tile_pool(name="pso", bufs=2, space="PSUM"))
             psa = p2.enter_context(tc.tile_pool(name="psa", bufs=1, space="PSUM"))
             psd = p2.enter_context(tc.tile_pool(name="psd", bufs=1, space="PSUM"))

             def oproj(attnT, q0):
                 # output projection for a finished group — issued one group
                 # late so its (ready) matmuls fill PE gaps while the next
                 # group's softmax chain is still in flight.
                 for tt in range(4):
                     r0 = q0 + tt * 128
                     ost = osp.tile([128, C], BF16, tag="ost")
                     for oc in range(4):
                         pO = pso.tile([128, 512], F32, tag="pso")
                         for h in range(HL):
                             nc.tensor.matmul(
                                 pO[:], attnT[h][:, tt * 128:(tt + 1) * 128],
                                 wo_sb[:, h * C + oc * 512: h * C + oc * 512 + 512],
                                 start=(h == 0), stop=(h == HL - 1))
                         nc.vector.tensor_copy(
                             ost[:, oc * 512:(oc + 1) * 512], pO[:])
                     if small_out:
                         nc.scalar.dma_start(out[0:128, :], ost[:])
                     else:
                         nc.scalar.dma_start(out[r0:r0 + 128, :], ost[:])

             pending = None
             for b in range(B):
                 for j in range(T // 512):   # q-chunks of 512 within the batch
                     q0 = b * T + j * 512
                     nkt = 4 * j + 4
                     npair = nkt // 2
                     # diagonal (masked) pairs FIRST: their exp->mask->PV
                     # chain is longest; later unmasked pairs keep the PE fed
                     # while it completes.
                     porder = [2 * j, 2 * j + 1] + list(range(2 * j))
                     attnT = []
                     for h in range(HL):
                         pA = psa.tile([128, 512], F32, tag="psa")
                         pDen = psd.tile([128, 512], F32, tag="psd")
                         for pi, p_ in enumerate(porder):
                             pS = psw.tile([128, 1024], F32, tag="psw")
                             ptile = ptp.tile([128, 1024], BF16, tag="ptile")
                             for half in range(2):
                                 i = 2 * p_ + half
                                 g = b * (T // 128) + i
                                 nc.tensor.matmul(
                                     pS[:, half * 512:(half + 1) * 512],
                                     kT[h][:, g * 128:(g + 1) * 128],
                                     qT[h][:, q0:q0 + 512], start=True, stop=True)
                             nc.scalar.activation(
                                 ptile[:], pS[:],
                                 mybir.ActivationFunctionType.Exp, scale=SCALE)
                             if 2 * p_ >= 4 * j and "nomask" not in ablate:
                                 nc.vector.tensor_mul(
                                     ptile[:], ptile[:], pairmasks[p_ - 2 * j][:])
                             for half in range(2):
                                 g = b * (T // 128) + 2 * p_ + half
                                 pt_h = ptile[:, half * 512:(half + 1) * 512]
                                 if "nopv" not in ablate:
                                     nc.tensor.matmul(
                                         pA[:],
                                         v_all[:, g * DH + h * 128: g * DH + (h + 1) * 128],
                                         pt_h, start=(pi == 0 and half == 0),
                                         stop=(pi == npair - 1 and half == 1))
                             # pre-add the two halves -> ONE den matmul per pair
                             pds = rdp.tile([128, 512], BF16, tag="pds")
                             nc.vector.tensor_add(
                                 pds[:], ptile[:, 0:512], ptile[:, 512:1024])
                             nc.tensor.matmul(
                                 pDen[:], ones_mat[:], pds[:],
                                 start=(pi == 0), stop=(pi == npair - 1))
                         bc = rdp.tile([128, 512], F32, tag="bc")
                         nc.vector.reciprocal(bc[:], pDen[:])
                         aT = atp.tile([128, 512], BF16, tag=f"aT{h}")
                         nc.vector.tensor_mul(aT[:], pA[:], bc[:])
                         attnT.append(aT)

                     if stop_after == "attn":
                         ost = osp.tile([128, C], F32, tag="ost", name="osta")
                         nc.vector.tensor_copy(ost[:, 0:512], attnT[0][:])
                         nc.vector.tensor_copy(ost[:, 512:1024], attnT[1][:])
                         nc.sync.dma_start(out[0:128, 0:1024], ost[:, 0:1024])
                         continue
                     if pending is not None:
                         oproj(*pending)
                     pending = (attnT, q0)
             if pending is not None and stop_after != "attn":
                 oproj(*pending)

    nc.compile()
    return nc


def _get_nc():
    if "nc" not in _CACHE:
        _CACHE["nc"] = _build()
    return _CACHE["nc"]


def _trig_tiles(freqs_cos, freqs_sin):
    cos = np.asarray(freqs_cos, np.float32)   # [T, 64]
    sin = np.asarray(freqs_sin, np.float32)
    jrow = np.array([16 * (p // 32) + p % 16 for p in range(128)])
    sign = np.array([-1.0 if p % 32 < 16 else 1.0 for p in range(128)],
                    np.float32)
    ct = np.ascontiguousarray(cos.T[jrow])
    st = np.ascontiguousarray(sin.T[jrow] * sign[:, None])
    return ct, st


def kernel(x, wq, wk, wv, wo, freqs_cos, freqs_sin, mask=None, **_unused):
    bf = ml_dtypes.bfloat16
    nc = _get_nc()

    x = np.asarray(x, dtype=np.float32)
    xt = np.ascontiguousarray(x.reshape(TT, C).T).astype(bf)
    ct, st = _trig_tiles(freqs_cos, freqs_sin)
    perm_local = np.concatenate([h * 128 + RPERM for h in range(HL)])

    in_maps = []
    for i in range(NCORES):
        sl = slice(DH * i, DH * (i + 1))
        wq_l = np.asarray(wq, np.float32)[sl, :][perm_local, :]
        wk_l = np.asarray(wk, np.float32)[sl, :][perm_local, :]
        in_maps.append({
            "xt": xt,
            "wqt": np.ascontiguousarray(wq_l.T).astype(bf),
            "wkt": np.ascontiguousarray(wk_l.T).astype(bf),
            "wvt": np.ascontiguousarray(np.asarray(wv, np.float32)[sl, :].T).astype(bf),
            "wot": np.ascontiguousarray(np.asarray(wo, np.float32)[:, sl].T).astype(bf),
            "ct": ct,
            "st": st,
        })

    res = run_bass_kernel_spmd(nc, in_maps, core_ids=list(range(NCORES)))
    acc = np.zeros((TT, C), dtype=np.float32)
    for r in res.results:
        acc += np.asarray(r["out"], dtype=np.float32)
    return acc.reshape(B, T, C)
